# revision 33
# baseline (speedup 1.0000x reference)
"""B3-spline undecimated wavelet transform (3 levels, reflect BC) on 8 trn2 cores.

Strategy
--------
Pure data parallel: 16 images -> 2 images per core.

Per level the separable 5-tap conv y = K_d @ Y @ K_d^T is computed as two
TensorEngine passes that each convolve along the *partition* axis and
transpose "for free":

    pass1:  AT = (K @ Y)^T      matmul(lhsT=Y_block, rhs=K^T_block)
    pass2:  Ynew = (K @ AT)^T   matmul(lhsT=AT_block, rhs=K^T_block)

K_d is banded (halfwidth 2d <= 8), so for each 128-row contraction block cb
only a narrow output window [cb*128-hw, cb*128+128+hw) is nonzero; each
window is issued as 1-2 matmuls (split at the 512-col PSUM bank boundary)
accumulating into a [128,1024] PSUM tile via the per-element has_written
bits. All matmul inputs are fp16: the conv weights are dyadic rationals and
exact in fp16, accumulation is fp32 in PSUM, and the wavelet subtraction
w = Y - Ynew runs on fp32 PSUM data, so end-to-end error is ~5e-4.

DVE/ACT do the PSUM evacuations (cast to fp16 for the next pass) and the
subtractions; HWDGE DMA streams the 4 output planes per image back to HBM.
"""

import sys

if "/opt/trn_rl_repo" not in sys.path:
    sys.path.insert(0, "/opt/trn_rl_repo")

import numpy as np

import concourse.bass as bass
import concourse.mybir as mybir
import concourse.tile as tile
from concourse import bacc
from concourse.bass_utils import run_bass_kernel_spmd

P = 128
L = 1024
NB = L // P            # 8 blocks per axis
BPC = 2                # images per core
NCORES = 8
LEVELS = (1, 2, 4)     # dilation per level
F32 = mybir.dt.float32
F16 = mybir.dt.float16
W5 = (1.0 / 16, 1.0 / 4, 3.0 / 8, 1.0 / 4, 1.0 / 16)
EVAC_SPLIT = 0  # 0: whole-tile evac copies (2 DVE / 6 ACT); else split column
CAST_ENGINE = "vector"  # engine for the fp32->fp16 input cast
L3_STORE_BLOCKS = 2  # h-blocks per last-level store DMA (1, 2, or 4)
EVAC_ALT = True  # alternate pass1 DVE evac set between levels


def _conv_matrix(d: int) -> np.ndarray:
    """K such that (K @ x) == dilated reflect-padded 5-tap conv along axis 0."""
    eye = np.eye(L, dtype=np.float64)
    xp = np.pad(eye, ((2 * d, 2 * d), (0, 0)), mode="reflect")
    K = np.zeros((L, L), dtype=np.float64)
    for k in range(5):
        K += W5[k] * xp[k * d : k * d + L]
    return K.astype(np.float32)


def _const_arrays() -> dict[str, np.ndarray]:
    """fp16 K^T blocks per level: interior Toeplitz block + the two edge blocks."""
    consts = {}
    for li, d in enumerate(LEVELS):
        hw = 2 * d
        KT = _conv_matrix(d).T  # KT[i, n] = K[n, i]
        kint = KT[P : 2 * P, P - hw : 2 * P + hw]
        k0 = KT[0:P, 0 : P + hw]
        k7 = KT[7 * P : 8 * P, 7 * P - hw : 8 * P]
        for nm, a in ((f"kint{li}", kint), (f"k0{li}", k0), (f"k7{li}", k7)):
            a16 = np.ascontiguousarray(a, dtype=np.float16)
            assert np.array_equal(a16.astype(np.float32), a.astype(np.float32))
            consts[nm] = a16
    return consts


def _windows(li: int, cb: int):
    """Nonzero output-column segments for contraction block cb, split at the
    PSUM bank boundary. Returns [(c0, c1, const_name, rhs_col_offset)]."""
    hw = 2 * LEVELS[li]
    if cb == 0:
        c0, c1, nm, base = 0, P + hw, f"k0{li}", 0
    elif cb == NB - 1:
        c0, c1, nm, base = 7 * P - hw, L, f"k7{li}", 7 * P - hw
    else:
        c0, c1, nm, base = cb * P - hw, cb * P + P + hw, f"kint{li}", cb * P - hw
    segs = [(c0, 512), (512, c1)] if c0 < 512 < c1 else [(c0, c1)]
    return [(a, b, nm, a - base) for a, b in segs]


def _mm_list(li: int):
    """Ordered matmul segments for one PSUM tile with per-bank start/stop."""
    segs = []
    for cb in range(NB):
        for a, b, nm, off in _windows(li, cb):
            segs.append([cb, a, b, nm, off, False, False])
    first, last = {}, {}
    for i, s in enumerate(segs):
        bank = s[1] // 512
        first.setdefault(bank, i)
        last[bank] = i
    for i in first.values():
        segs[i][5] = True  # start: clears the bank's has_written bits
    for i in last.values():
        segs[i][6] = True  # stop: closes the accumulation group
    return [tuple(s) for s in segs]


def _conv_pass(nc, ksb, src_tiles, segs, pspool, consume):
    """One transposing conv pass: 8 src tiles [P, L] fp16 -> 8 PSUM tiles [P, L]."""
    for mb in range(NB):
        ps = pspool.tile([P, L], F32, tag="ps", name="ps")
        for cb, a, b, nm, off, st, sp in segs:
            nc.tensor.matmul(
                ps[:, a:b],
                src_tiles[cb][:, mb * P : (mb + 1) * P],
                ksb[nm][:, off : off + (b - a)],
                start=st,
                stop=sp,
            )
        consume(mb, ps)


def _build_nc(repeat: int = 1):
    consts = _const_arrays()
    nc = bacc.Bacc(
        "TRN2",
        target_bir_lowering=False,
        debug=False,
        num_devices=NCORES,
    )
    x_in = nc.dram_tensor("x", [BPC, L, L], F32, kind="ExternalInput")
    out = nc.dram_tensor("out", [BPC, 4, L, L], F32, kind="ExternalOutput")
    knames = list(consts)
    kwidths = [consts[nm].shape[1] for nm in knames]
    koffs = dict(zip(knames, np.cumsum([0] + kwidths[:-1]).tolist()))
    ktotal = int(sum(kwidths))
    kall = nc.dram_tensor("kall", [P, ktotal], F16, kind="ExternalInput")

    with tile.TileContext(nc) as tc:
        with (
            tc.tile_pool(name="consts", bufs=1) as cpool,
            tc.tile_pool(name="xin", bufs=2 * NB) as xpool,
            tc.tile_pool(name="f16", bufs=2 * NB) as fpool,
            tc.tile_pool(name="wout", bufs=4) as wpool,
            tc.tile_pool(name="ps", bufs=4, space="PSUM") as pspool,
        ):
            kall_sb = cpool.tile([P, ktotal], F16, name="kall_sb")
            nc.scalar.dma_start(kall_sb[:], kall[:, :])
            ksb = {
                nm: kall_sb[:, koffs[nm] : koffs[nm] + consts[nm].shape[1]]
                for nm in knames
            }

            for img in [i % BPC for i in range(repeat * BPC)]:
                # x: per-block loads + casts so PE can start as data arrives
                x_tiles, cur = [], []
                for b in range(NB):
                    xt = xpool.tile([P, L], F32, tag="x", name="x_sb")
                    nc.scalar.dma_start(xt[:], x_in[img, b * P : (b + 1) * P])
                    ct = fpool.tile([P, L], F16, tag="cur", name="cur")
                    getattr(nc, CAST_ENGINE).tensor_copy(ct[:], xt[:])
                    x_tiles.append(xt)
                    cur.append(ct)

                for li in range(len(LEVELS)):
                    segs = _mm_list(li)
                    last = li == len(LEVELS) - 1

                    # pass 1: AT = (K @ Y)^T, evacuated to fp16 per block
                    at = [
                        fpool.tile([P, L], F16, tag="at", name="at")
                        for _ in range(NB)
                    ]

                    # early blocks evac on DVE: the LATE evacs gate the next
                    # pass's PSUM slot reuse, so they ride the faster ACT path
                    dve_mbs = (0, 1) if (li % 2 == 0 or not EVAC_ALT) else (0, 1, 2)

                    def evac_at(mb, ps, at=at, dve_mbs=dve_mbs):
                        if EVAC_SPLIT:
                            # split so neither engine paces the pass
                            nc.vector.tensor_copy(
                                at[mb][:, 0:EVAC_SPLIT], ps[:, 0:EVAC_SPLIT]
                            )
                            nc.scalar.copy(
                                at[mb][:, EVAC_SPLIT:L], ps[:, EVAC_SPLIT:L]
                            )
                        elif mb in dve_mbs:
                            nc.vector.tensor_copy(at[mb][:, :], ps[:, :])
                        else:
                            nc.scalar.copy(at[mb][:, :], ps[:, :])

                    _conv_pass(nc, ksb, cur, segs, pspool, evac_at)

                    # pass 2: Ynew = (K @ AT)^T; w = carrier - Ynew on DVE,
                    # Ynew cast fp16 on ACT for the next level (fp32 c_J on
                    # the last level). Output staged in half-image tiles so
                    # stores start at the half-way point.
                    w_halves = [
                        wpool.tile([P, NB // 2, L], F32, tag="w", name="w_sb")
                        for _ in range(2)
                    ]
                    c3_halves = (
                        [
                            wpool.tile([P, NB // 2, L], F32, tag="w", name="c3_sb")
                            for _ in range(2)
                        ]
                        if last
                        else None
                    )
                    nxt = (
                        None
                        if last
                        else [
                            fpool.tile([P, L], F16, tag="cur", name="nxt")
                            for _ in range(NB)
                        ]
                    )
                    carrier = x_tiles if li == 0 else cur

                    def evac_y(
                        mb, ps, w=w_halves, nxt=nxt, c3=c3_halves, carrier=carrier
                    ):
                        h, r = divmod(mb, NB // 2)
                        nc.vector.tensor_sub(
                            w[h][:, r, :], carrier[mb][:, :], ps[:, :]
                        )
                        if nxt is not None:
                            nc.scalar.copy(nxt[mb][:, :], ps[:, :])
                        else:
                            nc.scalar.copy(c3[h][:, r, :], ps[:, :])

                    _conv_pass(nc, ksb, at, segs, pspool, evac_y)

                    half = P * NB // 2
                    if not last:
                        for h in range(2):
                            nc.sync.dma_start(
                                out[img, li, h * half : (h + 1) * half].rearrange(
                                    "(b p) w -> p b w", p=P
                                ),
                                w_halves[h][:],
                            )
                    else:
                        # last level: finer-granularity stores so earlier
                        # blocks stream while later blocks still compute; c3
                        # rides the ACT HWDGE ring in parallel with w3.
                        g = L3_STORE_BLOCKS
                        for h in range(2):
                            for q in range(NB // 2 // g):
                                qi = NB // 2 // g * h + q
                                dst = slice(qi * P * g, (qi + 1) * P * g)
                                src = w_halves[h][:, q * g : (q + 1) * g, :]
                                c3s = c3_halves[h][:, q * g : (q + 1) * g, :]
                                if g > 1:
                                    dst_ap_w = out[img, li, dst].rearrange(
                                        "(b p) w -> p b w", p=P
                                    )
                                    dst_ap_c = out[img, 3, dst].rearrange(
                                        "(b p) w -> p b w", p=P
                                    )
                                else:
                                    dst_ap_w = out[img, li, dst]
                                    dst_ap_c = out[img, 3, dst]
                                    src = w_halves[h][:, q, :]
                                    c3s = c3_halves[h][:, q, :]
                                nc.sync.dma_start(dst_ap_w, src)
                                nc.scalar.dma_start(dst_ap_c, c3s)
                    cur = nxt
    nc.compile()
    return nc


def _kall_array() -> np.ndarray:
    consts = _const_arrays()
    return np.ascontiguousarray(
        np.concatenate([consts[nm] for nm in consts], axis=1), dtype=np.float16
    )


_NC_CACHE = None


def _get_nc():
    global _NC_CACHE
    if _NC_CACHE is None:
        _NC_CACHE = _build_nc()
    return _NC_CACHE


def _run(x: np.ndarray, **spmd_kwargs):
    x = np.ascontiguousarray(x, dtype=np.float32)
    assert x.shape == (BPC * NCORES, L, L), x.shape
    nc = _get_nc()
    kall = _kall_array()
    in_maps = []
    for c in range(NCORES):
        m = {
            "x": np.ascontiguousarray(x[c * BPC : (c + 1) * BPC]),
            "kall": kall,
        }
        in_maps.append(m)
    res = run_bass_kernel_spmd(nc, in_maps, core_ids=list(range(NCORES)), **spmd_kwargs)
    full = np.concatenate([res.results[c]["out"] for c in range(NCORES)], axis=0)
    return full, res


def kernel(x: np.ndarray) -> np.ndarray:
    full, _ = _run(x)
    return full
